# revision 2
# baseline (speedup 1.0000x reference)
"""DARTS-cell (moe_routing) Trainium2 kernel.

Strategy: data-parallel over batch B=32 across 8 cores (4 samples/core).
Per-sample top-2-of-8 gating (alphas) computed on host; zero-alpha branches
contribute exactly zero (dense mode) or are skipped (sparse mode).

Layout on device: channel-major [C=128 partitions, H*W=1024 free] per sample.
- 1x1 convs + preprocess: PE matmuls in float32r (full-rate fp32, ~12-bit
  mantissa rounding) accumulating into per-(step,sample) PSUM state.
- depthwise convs: per-partition-scalar shifted MACs (scalar_tensor_tensor)
  on DVE/GpSimd over zero-padded relu'd buffers built by ScalarE
  (activation Relu with per-sample alpha folded into the scale).
- pools: shifted tensor_max/tensor_add trees; avg uses a precomputed
  BN/count map; pool results + skip are added into state PSUM by DVE after
  all matmuls (PSUM has_written rule).
- BN (eval, affine=False) folded into weights/maps on host.
"""

import sys

sys.path.insert(0, "/opt/trn_rl_repo")

import numpy as np
from concourse import bacc, mybir, tile
from concourse.bass_utils import run_bass_kernel_spmd

STEPS = 4
N_MIX = 14
OFFSETS = [0, 2, 5, 9]
B, C_IN, C, H, W = 32, 512, 128, 32, 32
HW = H * W
N_CORES = 8
BL = B // N_CORES  # samples per core
BN_SCALE = float(1.0 / np.sqrt(1.0 + 1e-5))

F32 = mybir.dt.float32
F32R = mybir.dt.float32r
ALU = mybir.AluOpType
ACTF = mybir.ActivationFunctionType

# branch op indices in PRIMITIVES order
O_MAX, O_AVG, O_SKIP, O_SEP3, O_SEP5, O_DIL3, O_DIL5 = 1, 2, 3, 4, 5, 6, 7

# dw tap table layout per mixed-op m: [sep3_dw1(9), sep3_dw2(9), sep5_dw1(25),
# sep5_dw2(25), dil3(9), dil5(25)] -> 102 taps
TAP_OFF = {"s3a": 0, "s3b": 9, "d3": 18}
N_TAPS = 27
# pw matrix slots per m (DVE branches only); sep5/dil5 are PE-fused
PW_SLOT = {"s3a": 0, "s3b": 1, "d3": 2}
N_PW = 3


def _host_alphas(gates, top):
    """Per-sample masked-softmax over top-k gate entries. gates [N_MIX,B,8]."""
    g = gates.astype(np.float64)
    idx = np.argsort(-g, axis=-1, kind="stable")[..., :top]  # [m,b,top]
    mask = np.zeros(g.shape, bool)
    np.put_along_axis(mask, idx, True, axis=-1)
    gm = np.where(mask, g, -np.inf)
    gm -= gm.max(axis=-1, keepdims=True)
    e = np.exp(gm)
    p = e / e.sum(axis=-1, keepdims=True)
    return p.astype(np.float32)  # exact zeros off top-k


def build_program(active, n_cores=N_CORES):
    """active[(m, b_local)] -> iterable of branch op indices (1..7) to emit.
    Must be the same for every core (SPMD); dense mode passes all 7."""
    nc = bacc.Bacc("TRN2", target_bir_lowering=False, debug=False,
                   num_devices=n_cores)

    x0_d = nc.dram_tensor("x0", [BL, 4, 128, HW], F32, kind="ExternalInput").ap()
    x1_d = nc.dram_tensor("x1", [BL, 4, 128, HW], F32, kind="ExternalInput").ap()
    prew_d = nc.dram_tensor("prew", [128, 2, 4, 128], F32R, kind="ExternalInput").ap()
    pw_d = nc.dram_tensor("pw", [128, N_MIX, N_PW, 128], F32R, kind="ExternalInput").ap()
    fw5a_d = nc.dram_tensor("fw5a", [128, N_MIX, 25, 128], F32R, kind="ExternalInput").ap()
    fw5b_d = nc.dram_tensor("fw5b", [128, N_MIX, 25, 128], F32R, kind="ExternalInput").ap()
    fwd5_d = nc.dram_tensor("fwd5", [128, N_MIX, 25, 128], F32R, kind="ExternalInput").ap()
    dwt_d = nc.dram_tensor("dwt", [128, N_MIX, N_TAPS], F32, kind="ExternalInput").ap()
    alf_d = nc.dram_tensor("alf", [128, N_MIX, BL, 8], F32, kind="ExternalInput").ap()
    rmap_d = nc.dram_tensor("rmap", [128, 32, 32], F32, kind="ExternalInput").ap()
    out_d = nc.dram_tensor("out", [BL, 4, 128, HW], F32R, kind="ExternalOutput").ap()

    with tile.TileContext(nc) as tc:
        with (
            tc.tile_pool(name="const", bufs=1) as cpool,
            tc.tile_pool(name="work", bufs=1) as wpool,
            tc.tile_pool(name="xs", bufs=2) as xpool,
            tc.tile_pool(name="dwa", bufs=2) as dpool,
            tc.tile_pool(name="ps_state", bufs=2, space="PSUM") as pspool,
            tc.tile_pool(name="ps_scr", bufs=2, space="PSUM") as scrpool,
            tc.tile_pool(name="fw", bufs=2) as fwpool,
        ):
            # ---- constants / weights ----
            prew = cpool.tile([128, 2, 4, 128], F32R, tag="prew")
            pw = cpool.tile([128, N_MIX, N_PW, 128], F32R, tag="pw")
            dwt = cpool.tile([128, N_MIX, N_TAPS], F32, tag="dwt")
            alf = cpool.tile([128, N_MIX, BL, 8], F32, tag="alf")
            rmap = cpool.tile([128, 32, 32], F32, tag="rmap")
            nc.sync.dma_start(prew[:], prew_d)
            nc.sync.dma_start(pw[:], pw_d)
            nc.sync.dma_start(dwt[:], dwt_d)
            nc.sync.dma_start(alf[:], alf_d)
            nc.sync.dma_start(rmap[:], rmap_d)

            # ---- persistent padded work buffers ----
            z34 = [wpool.tile([128, 34, 34], F32R, tag=f"z34_{i}", name=f"z34_{i}") for i in range(2)]
            z36 = [wpool.tile([128, 36, 36], F32R, tag=f"z36_{i}", name=f"z36_{i}") for i in range(2)]
            z40 = [wpool.tile([128, 40, 40], F32R, tag=f"z40_{i}", name=f"z40_{i}") for i in range(2)]
            xpmax = wpool.tile([128, 34, 34], F32, tag="xpmax")
            xpsum = wpool.tile([128, 34, 34], F32, tag="xpsum")
            rmpad = wpool.tile([128, 34, 32], F32, tag="rmpad")
            rspad = wpool.tile([128, 34, 32], F32, tag="rspad")
            ptmp = [wpool.tile([128, 32, 32], F32, tag=f"ptmp_{i}", name=f"ptmp_{i}") for i in range(2)]

            states = wpool.tile([128, 6, 32, 32], F32R, tag="states")
            pooled = wpool.tile([128, 2, 5, 32, 32], F32, tag="pooled")

            for z in z34 + z36 + z40:
                nc.gpsimd.memset(z[:].bitcast(F32), 0.0)
            nc.gpsimd.memset(xpmax[:], -1e30)
            nc.gpsimd.memset(xpsum[:], 0.0)
            nc.gpsimd.memset(rmpad[:], -1e30)
            nc.gpsimd.memset(rspad[:], 0.0)

            zpad_for = {  # branch -> (buffers, pad, tap stride)
                O_SEP3: (z34, 1, 1),
                O_SEP5: (z36, 2, 1),
                O_DIL3: (z40, 2, 2),
                O_DIL5: (z40, 4, 2),
            }

            def flat(ap3):  # [128, a, b] -> [128, a*b]
                return ap3.rearrange("p a b -> p (a b)")

            def mm_chunks(psum3, lhsT, rhs3, flags):
                """two N=512 matmuls; flags = (start0, stop0, start1, stop1)."""
                s0, e0, s1, e1 = flags
                nc.tensor.matmul(psum3[:, 0:16, :], lhsT, rhs3[:, 0:16, :],
                                 start=s0, stop=e0)
                nc.tensor.matmul(psum3[:, 16:32, :], lhsT, rhs3[:, 16:32, :],
                                 start=s1, stop=e1)

            def dw_chain(eng, zt, dwacc, m, tap0, k, pad, stride, interior):
                """depthwise conv: dwacc = sum_t dwt[:,m,tap0+t] * shift_t(zt)."""
                first = True
                for ky in range(k):
                    for kx in range(k):
                        t = tap0 + ky * k + kx
                        y0 = interior - pad + stride * ky
                        x0 = interior - pad + stride * kx
                        view = zt[:, y0:y0 + 32, x0:x0 + 32]
                        sc = dwt[:, m, t:t + 1]
                        if first:
                            eng.tensor_scalar_mul(dwacc[:], view, sc)
                            first = False
                        else:
                            eng.scalar_tensor_tensor(
                                dwacc[:], view, sc, dwacc[:],
                                op0=ALU.mult, op1=ALU.add)

            def fused_stage(fw_d, m, zt, pad, stride, interior, k, psum3,
                            gfirst, glast):
                """depthwise+pointwise fused: accumulate k*k tap matmuls."""
                taps = k * k
                half = (taps + 1) // 2
                for (a, e) in ((0, half), (half, taps)):
                    fwt = fwpool.tile([128, 13, 128], F32R, tag="fw")
                    nc.sync.dma_start(fwt[:, 0:e - a, :], fw_d[:, m, a:e, :])
                    for t in range(a, e):
                        ky, kx = divmod(t, k)
                        y0 = interior - pad + stride * ky
                        x0 = interior - pad + stride * kx
                        st = gfirst and t == 0
                        sp = glast and t == taps - 1
                        for h2 in range(2):
                            nc.tensor.matmul(
                                psum3[:, 16 * h2:16 * h2 + 16, :],
                                fwt[:, t - a, :],
                                zt[:, y0 + 16 * h2:y0 + 16 * h2 + 16,
                                   x0:x0 + 32],
                                start=st, stop=sp)

            def conv_branch(o, m, b, x3, stp, stp_flags):
                """emit one conv branch. stp_flags = (gfirst, glast) for the
                state-psum accumulation group."""
                gfirst, glast = stp_flags
                if o == O_SEP5:  # PE-fused two-stage
                    z1 = z36[0]
                    nc.scalar.activation(z1[:, 2:34, 2:34], x3, ACTF.Relu,
                                         scale=alf[:, m, b, o:o + 1])
                    scr = scrpool.tile([128, 32, 32], F32, tag="scr")
                    fused_stage(fw5a_d, m, z1, 2, 1, 2, 5, scr, True, True)
                    z2 = z36[1]
                    nc.scalar.activation(z2[:, 2:34, 2:34], scr[:], ACTF.Relu)
                    fused_stage(fw5b_d, m, z2, 2, 1, 2, 5, stp, gfirst, glast)
                    return
                if o == O_DIL5:  # PE-fused one-stage
                    z1 = z40[0]
                    nc.scalar.activation(z1[:, 4:36, 4:36], x3, ACTF.Relu,
                                         scale=alf[:, m, b, o:o + 1])
                    fused_stage(fwd5_d, m, z1, 4, 2, 4, 5, stp, gfirst, glast)
                    return
                if o == O_SEP3:
                    bufs, pad, stride, k, t0a, t0b = z34, 1, 1, 3, TAP_OFF["s3a"], TAP_OFF["s3b"]
                    pwa, pwb = PW_SLOT["s3a"], PW_SLOT["s3b"]
                else:  # O_DIL3
                    bufs, pad, stride, k, t0a = z40, 2, 2, 3, TAP_OFF["d3"]
                    pwa = PW_SLOT["d3"]
                interior = (bufs[0].shape[1] - 32) // 2
                i0, i1 = interior, interior + 32
                two_stage = o == O_SEP3
                eng = nc.vector  # scalar_tensor_tensor not supported on GpSimd

                z1 = bufs[0]
                nc.scalar.activation(z1[:, i0:i1, i0:i1], x3, ACTF.Relu,
                                     scale=alf[:, m, b, o:o + 1])
                dwacc = dpool.tile([128, 32, 32], F32R, tag="dwacc")
                dw_chain(eng, z1, dwacc, m, t0a, k, pad, stride, interior)
                if two_stage:
                    scr = scrpool.tile([128, 32, 32], F32, tag="scr")
                    mm_chunks(scr, pw[:, m, pwa, :], dwacc,
                              (True, True, True, True))
                    z2 = bufs[1]
                    nc.scalar.activation(z2[:, i0:i1, i0:i1], scr[:], ACTF.Relu)
                    dwacc2 = dpool.tile([128, 32, 32], F32R, tag="dwacc")
                    dw_chain(eng, z2, dwacc2, m, t0b, k, pad, stride, interior)
                    mm_chunks(stp, pw[:, m, pwb, :], dwacc2,
                              (gfirst, glast, gfirst, glast))
                else:
                    mm_chunks(stp, pw[:, m, pwa, :], dwacc,
                              (gfirst, glast, gfirst, glast))

            def build_pools(j, need_max, need_avg):
                """pool state j -> pooled[:,0,j] (max), pooled[:,1,j] (BN*avg)."""
                x3 = states[:, j]
                if need_max:
                    nc.scalar.copy(xpmax[:, 1:33, 1:33], x3)
                    t = ptmp[0]
                    nc.vector.tensor_max(t[:], xpmax[:, 1:33, 0:32],
                                         xpmax[:, 1:33, 1:33])
                    nc.vector.tensor_max(rmpad[:, 1:33, :], t[:],
                                         xpmax[:, 1:33, 2:34])
                    nc.vector.tensor_max(t[:], rmpad[:, 0:32, :],
                                         rmpad[:, 1:33, :])
                    nc.vector.tensor_max(pooled[:, 0, j], t[:],
                                         rmpad[:, 2:34, :])
                if need_avg:
                    nc.scalar.copy(xpsum[:, 1:33, 1:33], x3)
                    t = ptmp[1]
                    nc.gpsimd.tensor_add(t[:], xpsum[:, 1:33, 0:32],
                                         xpsum[:, 1:33, 1:33])
                    nc.gpsimd.tensor_add(rspad[:, 1:33, :], t[:],
                                         xpsum[:, 1:33, 2:34])
                    nc.gpsimd.tensor_add(t[:], rspad[:, 0:32, :],
                                         rspad[:, 1:33, :])
                    nc.gpsimd.tensor_add(pooled[:, 1, j], t[:],
                                         rspad[:, 2:34, :])
                    nc.gpsimd.tensor_mul(pooled[:, 1, j], pooled[:, 1, j],
                                         rmap[:])

            # which (m,b) use pools, per source state j
            def pool_needs(j, b):
                nm = nav = False
                for step in range(STEPS):
                    if j < 2 + step:
                        m = OFFSETS[step] + j
                        acts = active.get((m, b), ())
                        nm |= O_MAX in acts
                        nav |= O_AVG in acts
                return nm, nav

            # ================= per-sample program =================
            for b in range(BL):
                # ---- preprocess s0, s1 ----
                for inp, xd in ((0, x0_d), (1, x1_d)):
                    scr = scrpool.tile([128, 32, 32], F32, tag="scr")
                    for kc in range(4):
                        xb = xpool.tile([128, HW], F32, tag="xb")
                        nc.sync.dma_start(xb[:], xd[b, kc])
                        xr = xpool.tile([128, HW], F32R, tag="xr")
                        nc.scalar.activation(xr[:], xb[:], ACTF.Relu)
                        for h in range(2):
                            nc.tensor.matmul(
                                scr[:, 16 * h:16 * (h + 1), :],
                                prew[:, inp, kc, :],
                                xr[:, 512 * h:512 * (h + 1)].rearrange(
                                    "p (a c) -> p a c", a=16),
                                start=(kc == 0), stop=(kc == 3))
                    nc.scalar.copy(states[:, inp], scr[:])

                for j in range(2):
                    nm, nav = pool_needs(j, b)
                    build_pools(j, nm, nav)

                # ---- steps ----
                for step in range(STEPS):
                    n_in = 2 + step
                    m0 = OFFSETS[step]
                    stp = pspool.tile([128, 32, 32], F32, tag="stp")
                    # count final matmuls per chunk to set start/stop flags
                    conv_list = []
                    post_list = []
                    for j in range(n_in):
                        m = m0 + j
                        for o in active.get((m, b), ()):
                            if o in (O_SEP3, O_SEP5, O_DIL3, O_DIL5):
                                conv_list.append((o, m, j))
                            else:
                                post_list.append((o, m, j))
                    n_mm = len(conv_list)
                    for i, (o, m, j) in enumerate(conv_list):
                        flags = (i == 0, i == n_mm - 1)
                        conv_branch(o, m, b, states[:, j], stp, flags)
                    if n_mm == 0:
                        nc.vector.memset(stp[:], 0.0)
                    for (o, m, j) in post_list:
                        if o == O_SKIP:
                            src = states[:, j]
                            sc = alf[:, m, b, O_SKIP:O_SKIP + 1]
                        elif o == O_MAX:
                            src = pooled[:, 0, j]
                            sc = alf[:, m, b, O_MAX:O_MAX + 1]
                        else:
                            src = pooled[:, 1, j]
                            sc = alf[:, m, b, O_AVG:O_AVG + 1]
                        nc.vector.scalar_tensor_tensor(
                            stp[:], src, sc, stp[:], op0=ALU.mult, op1=ALU.add)
                    # evacuate state
                    nc.scalar.copy(states[:, 2 + step], stp[:])
                    if step < STEPS - 1:
                        nm, nav = pool_needs(2 + step, b)
                        build_pools(2 + step, nm, nav)

                # ---- output ----
                for i in range(4):
                    nc.sync.dma_start(out_d[b, i],
                                      flat(states[:, 2 + i]))

    nc.compile()
    return nc


def host_prepare(inputs):
    """Returns (in_maps, alphas). in_maps: per-core input dicts."""
    s0, s1 = np.asarray(inputs["s0"]), np.asarray(inputs["s1"])
    gates = np.asarray(inputs["gates"])
    top = int(inputs["top"])
    p = _host_alphas(gates, top)  # [N_MIX, B, 8] fp32, exact zeros

    # prew [128, 2, 4, 128]: prew[ci_local, inp, kc, co] = w[co, kc*128+ci] * BN
    prew = np.empty((128, 2, 4, 128), np.float32)
    for inp, wname in ((0, "pre0_w"), (1, "pre1_w")):
        wmat = np.asarray(inputs[wname]) * BN_SCALE  # [C, C_in]
        for kc in range(4):
            prew[:, inp, kc, :] = wmat[:, 128 * kc:128 * (kc + 1)].T

    # pw [128, N_MIX, N_PW, 128]: pw_out[ci, m, slot, co] = w[m, co, ci] * BN
    pw = np.empty((128, N_MIX, N_PW, 128), np.float32)
    for nm, key in (("s3a", "sep3_pw1"), ("s3b", "sep3_pw2"),
                    ("d3", "dil3_pw")):
        wmat = np.asarray(inputs[key]).astype(np.float32) * BN_SCALE  # [M,Co,Ci]
        pw[:, :, PW_SLOT[nm], :] = wmat.transpose(2, 0, 1)

    # fused tap matrices for PE branches: fw[ci, m, t, co] = pw[m,co,ci]*dw[m,ci,t]
    def fuse(pw_key, dw_key, k):
        pwm = np.asarray(inputs[pw_key]).astype(np.float32) * BN_SCALE  # [M,Co,Ci]
        dwm = np.asarray(inputs[dw_key]).astype(np.float32).reshape(N_MIX, C, k * k)
        pwT = pwm.transpose(2, 0, 1)  # [Ci, M, Co]
        dwT = dwm.transpose(1, 0, 2)  # [Ci, M, taps]
        return (pwT[:, :, None, :] * dwT[:, :, :, None]).astype(np.float32)

    fw5a = fuse("sep5_pw1", "sep5_dw1", 5)
    fw5b = fuse("sep5_pw2", "sep5_dw2", 5)
    fwd5 = fuse("dil5_pw", "dil5_dw", 5)

    # dwt [128, N_MIX, 102]: dwt[c, m, tap]
    dwt = np.empty((128, N_MIX, N_TAPS), np.float32)
    for nm, key, k in (("s3a", "sep3_dw1", 3), ("s3b", "sep3_dw2", 3),
                       ("d3", "dil3_dw", 3)):
        w = np.asarray(inputs[key])  # [N_MIX, C, k, k]
        dwt[:, :, TAP_OFF[nm]:TAP_OFF[nm] + k * k] = (
            w.reshape(N_MIX, C, k * k).transpose(1, 0, 2))

    # rmap: BN * 9 / count (pools computed as straight 3x3 valid-sum)
    cnt = np.zeros((32, 32), np.float32)
    for dy in (-1, 0, 1):
        for dx in (-1, 0, 1):
            ys = slice(max(0, -dy), 32 - max(0, dy))
            cnt[max(0, dy):32 - max(0, -dy),
                max(0, dx):32 - max(0, -dx)] += 1
    rmap_1 = (BN_SCALE / cnt).astype(np.float32)
    rmap = np.broadcast_to(rmap_1, (128, 32, 32)).copy()

    in_maps = []
    for core in range(N_CORES):
        sl = slice(core * BL, (core + 1) * BL)
        alf = p[:, sl, :].copy()  # [N_MIX, BL, 8]
        alf[:, :, O_MAX] *= BN_SCALE
        alf_b = np.broadcast_to(alf, (128,) + alf.shape).copy()
        in_maps.append({
            "x0": s0[sl].reshape(BL, 4, 128, HW).astype(np.float32),
            "x1": s1[sl].reshape(BL, 4, 128, HW).astype(np.float32),
            "prew": prew, "pw": pw, "dwt": dwt,
            "fw5a": fw5a, "fw5b": fw5b, "fwd5": fwd5,
            "alf": alf_b.astype(np.float32), "rmap": rmap,
        })
    return in_maps, p


_prog_cache = {}


def _get_dense_program():
    key = "dense"
    if key not in _prog_cache:
        active = {(m, b): (O_MAX, O_AVG, O_SKIP, O_SEP3, O_SEP5, O_DIL3, O_DIL5)
                  for m in range(N_MIX) for b in range(BL)}
        _prog_cache[key] = build_program(active)
    return _prog_cache[key]


def prepare_run(inputs):
    """test.py helper: returns (in_maps, compiled nc) for a timed run."""
    in_maps, _ = host_prepare(inputs)
    return in_maps, _get_dense_program()


def kernel(**inputs):
    in_maps, _ = host_prepare(inputs)
    nc = _get_dense_program()
    res = run_bass_kernel_spmd(nc, in_maps, core_ids=list(range(N_CORES)))
    out = np.empty((B, 512, H, W), np.float32)
    for core in range(N_CORES):
        o = res.results[core]["out"]  # [BL, 4, 128, HW]
        out[core * BL:(core + 1) * BL] = (
            o.reshape(BL, 512, H, W).astype(np.float32))
    return out



# revision 7
# speedup vs baseline: 1.4801x; 1.4801x over previous
"""DARTS-cell (moe_routing) Trainium2 kernel — sparse slot-grid version.

Data-parallel over batch B=32 across 8 cores (4 samples/core). Top-2-of-8
routing means only ~1.77 of 7 branches are active per (m, sample); instead
of computing all branches densely, the program is specialized at runtime to
a padded per-(sample-slot, step, branch-type) slot grid sized by the max
count over cores (SPMD: one program, per-core DATA selects the work).

- slot inputs are selected data-dependently with gpsimd.ap_gather from the
  per-sample state stack (indices host-packed per core),
- slot weights/alphas are host-gathered into per-slot DRAM tables,
- dummy (padding) slots run with alpha=0 / zero weights and contribute 0.

Branch forms: sep5/dil5 PE-fused (per-tap matmuls, dw x pw fused on host);
sep3/dil3 dw on DVE (scalar_tensor_tensor) + pw matmul. All branches share
one zero-padded z-buffer geometry [128,40,40] with interior at [4,36).
States are stored bf16 (halves SBUF; output path keeps f32 via psum evac).
BN (eval, affine=False) folded into weights/maps on host.
"""

import sys

sys.path.insert(0, "/opt/trn_rl_repo")

import numpy as np
from concourse import bacc, mybir, tile
from concourse.bass_utils import run_bass_kernel_spmd

STEPS = 4
N_MIX = 14
OFFSETS = [0, 2, 5, 9]
B, C_IN, C, H, W = 32, 512, 128, 32, 32
HW = H * W
N_CORES = 8
BL = B // N_CORES
BN_SCALE = float(1.0 / np.sqrt(1.0 + 1e-5))

F32 = mybir.dt.float32
F32R = mybir.dt.float32r
BF16 = mybir.dt.bfloat16
I16 = mybir.dt.int16
ALU = mybir.AluOpType
ACTF = mybir.ActivationFunctionType

SDT = BF16  # state storage dtype (gathered); flip to F32 if precision needs

O_MAX, O_AVG, O_SKIP, O_SEP3, O_SEP5, O_DIL3, O_DIL5 = 1, 2, 3, 4, 5, 6, 7
CONV_OPS = (O_SEP5, O_SEP3, O_DIL5, O_DIL3)  # round-robin emission order
CHEAP_OPS = (O_MAX, O_AVG, O_SKIP)
TYPE_NAME = {O_SEP3: "s3", O_SEP5: "s5", O_DIL3: "d3", O_DIL5: "d5",
             O_MAX: "max", O_AVG: "avg", O_SKIP: "skip"}
# (k, pad, stride/dilation) per conv type; z interior at [4, 36)
CONV_GEO = {O_SEP3: (3, 1, 1), O_SEP5: (5, 2, 1),
            O_DIL3: (3, 2, 2), O_DIL5: (5, 4, 2)}
TWO_STAGE = {O_SEP3, O_SEP5}
PE_FUSED = {O_SEP5, O_DIL5}

# host-side unit costs (us) for assignment balancing
COST_PE = {O_SEP5: 26.7, O_DIL5: 13.4}
COST_DVE = {O_SEP3: 19.3, O_DIL3: 9.6}
COST_CHEAP = {O_MAX: 6.6, O_AVG: 2.5, O_SKIP: 1.1}  # DVE / gpsimd-ish


def _host_alphas(gates, top):
    g = np.asarray(gates).astype(np.float64)
    idx = np.argsort(-g, axis=-1, kind="stable")[..., :top]
    mask = np.zeros(g.shape, bool)
    np.put_along_axis(mask, idx, True, axis=-1)
    gm = np.where(mask, g, -np.inf)
    gm -= gm.max(axis=-1, keepdims=True)
    e = np.exp(gm)
    p = e / e.sum(axis=-1, keepdims=True)
    return p.astype(np.float32)


# ---------------------------------------------------------------- planning

def _grid_cost(act, assign):
    """assign: [8][BL] sample ids -> (cost, K) where K[(b,s,o)] = slot count."""
    K = {}
    pe = dve = 0.0
    for s in range(STEPS):
        for bs in range(BL):
            for o in CONV_OPS + CHEAP_OPS:
                mx = 0
                for core in range(N_CORES):
                    smp = assign[core][bs]
                    c = sum(int(act[OFFSETS[s] + j, smp, o])
                            for j in range(2 + s))
                    if c > mx:
                        mx = c
                K[(bs, s, o)] = mx
                if o in COST_PE:
                    pe += mx * COST_PE[o]
                elif o in COST_DVE:
                    dve += mx * COST_DVE[o]
                else:
                    dve += mx * COST_CHEAP[o]
    cost = max(pe, dve) + 0.15 * (pe + dve)
    return cost, K


def _optimize_assignment(act, iters=600, seed=0):
    rng = np.random.default_rng(seed)
    # start: LPT on total conv cost
    w = np.zeros(B)
    for m in range(N_MIX):
        for b in range(B):
            w[b] += sum(COST_PE.get(o, 0) + COST_DVE.get(o, 0)
                        for o in CONV_OPS if act[m, b, o])
    order = np.argsort(-w)
    loads = [0.0] * N_CORES
    assign = [[] for _ in range(N_CORES)]
    for b in order:
        c = min((i for i in range(N_CORES) if len(assign[i]) < BL),
                key=lambda i: loads[i])
        assign[c].append(int(b))
        loads[c] += w[b]
    best_cost, _ = _grid_cost(act, assign)
    best = [list(a) for a in assign]
    # random swap local search
    cur = [list(a) for a in best]
    cur_cost = best_cost
    for it in range(iters):
        c1, c2 = rng.integers(0, N_CORES, 2)
        s1, s2 = rng.integers(0, BL, 2)
        if c1 == c2 and s1 == s2:
            continue
        cur[c1][s1], cur[c2][s2] = cur[c2][s2], cur[c1][s1]
        cost, _ = _grid_cost(act, cur)
        if cost <= cur_cost:
            cur_cost = cost
            if cost < best_cost:
                best_cost = cost
                best = [list(a) for a in cur]
        else:
            cur[c1][s1], cur[c2][s2] = cur[c2][s2], cur[c1][s1]
    _, K = _grid_cost(act, best)
    return best, K, best_cost


def build_plan(gates, top):
    p = _host_alphas(gates, top)  # [N_MIX, B, 8]
    act = p > 0
    assign, K, cost = _optimize_assignment(act)

    # emission schedule per (bs, s): conv slot type list (round-robin),
    # then cheap slot type list
    sched = {}
    for bs in range(BL):
        for s in range(STEPS):
            convs = []
            rem = {o: K[(bs, s, o)] for o in CONV_OPS}
            while any(rem.values()):
                for o in CONV_OPS:
                    if rem[o]:
                        convs.append(o)
                        rem[o] -= 1
            cheaps = []
            for o in CHEAP_OPS:
                cheaps += [o] * K[(bs, s, o)]
            sched[(bs, s)] = (tuple(convs), tuple(cheaps))

    # global slot numbering (same for all cores; order = emission order)
    n_stage = n_pw = n_dve = n_conv = n_cheap = n_wave = 0
    meta = []  # per (bs,s): dict with slot base indices
    for s in range(STEPS):
        for bs in range(BL):
            convs, cheaps = sched[(bs, s)]
            info = dict(conv0=n_conv, cheap0=n_cheap, wave0=n_wave,
                        stage0=n_stage, pw0=n_pw, dve0=n_dve)
            for o in convs:
                if o in PE_FUSED:
                    n_stage += 2 if o in TWO_STAGE else 1
                else:
                    n_pw += 2 if o in TWO_STAGE else 1
                    n_dve += 1
            n_conv += len(convs)
            n_cheap += len(cheaps)
            n_wave += (len(convs) + len(cheaps) + 1) // 2
            meta.append(((bs, s), info))
    meta = dict(meta)

    key = tuple(sorted(K.items()))
    return dict(p=p, act=act, assign=assign, K=K, sched=sched, meta=meta,
                key=key, cost=cost,
                n=dict(stage=max(n_stage, 1), pw=max(n_pw, 1),
                       dve=max(n_dve, 1), conv=max(n_conv, 1),
                       cheap=max(n_cheap, 1), wave=max(n_wave, 1)))


# ---------------------------------------------------------------- program

def build_program(plan, n_cores=N_CORES):
    sched, meta, n = plan["sched"], plan["meta"], plan["n"]
    nc = bacc.Bacc("TRN2", target_bir_lowering=False, debug=False,
                   num_devices=n_cores)

    x0_d = nc.dram_tensor("x0", [BL, 4, 128, HW], F32, kind="ExternalInput").ap()
    x1_d = nc.dram_tensor("x1", [BL, 4, 128, HW], F32, kind="ExternalInput").ap()
    prew_d = nc.dram_tensor("prew", [128, 2, 4, 128], F32R, kind="ExternalInput").ap()
    fw_d = nc.dram_tensor("fw", [128, n["stage"], 25, 128], F32R, kind="ExternalInput").ap()
    pw_d = nc.dram_tensor("pw", [128, n["pw"], 128], F32R, kind="ExternalInput").ap()
    dwt_d = nc.dram_tensor("dwt", [128, n["dve"], 18], F32, kind="ExternalInput").ap()
    alf_d = nc.dram_tensor("alf", [128, n["conv"]], F32, kind="ExternalInput").ap()
    alfc_d = nc.dram_tensor("alfc", [128, n["cheap"]], F32, kind="ExternalInput").ap()
    idx_d = nc.dram_tensor("idx", [128, n["wave"]], I16, kind="ExternalInput").ap()
    rmap_d = nc.dram_tensor("rmap", [128, 32, 32], F32, kind="ExternalInput").ap()
    out_d = nc.dram_tensor("out", [BL, 4, 128, HW], F32, kind="ExternalOutput").ap()

    with tile.TileContext(nc) as tc:
        with (
            tc.tile_pool(name="const", bufs=1) as cpool,
            tc.tile_pool(name="work", bufs=1) as wpool,
            tc.tile_pool(name="xs", bufs=2) as xpool,
            tc.tile_pool(name="stg", bufs=3) as spool,
            tc.tile_pool(name="dwa", bufs=3) as dpool,
            tc.tile_pool(name="pwb", bufs=2) as pwpool,
            tc.tile_pool(name="fw", bufs=2) as fwpool,
            tc.tile_pool(name="ost", bufs=2) as opool,
            tc.tile_pool(name="ps_state", bufs=2, space="PSUM") as pspool,
            tc.tile_pool(name="ps_scr", bufs=2, space="PSUM") as scrpool,
        ):
            # ---- constants ----
            prew = cpool.tile([128, 2, 4, 128], F32R, tag="prew")
            dwt = cpool.tile([128, n["dve"], 18], F32, tag="dwt")
            alf = cpool.tile([128, n["conv"]], F32, tag="alf")
            alfc = cpool.tile([128, n["cheap"]], F32, tag="alfc")
            idx = cpool.tile([128, n["wave"]], I16, tag="idx")
            rmap = cpool.tile([128, 32, 32], F32, tag="rmap")
            for t, d in ((prew, prew_d), (dwt, dwt_d), (alf, alf_d),
                         (alfc, alfc_d), (idx, idx_d), (rmap, rmap_d)):
                nc.sync.dma_start(t[:], d)

            # ---- persistent buffers ----
            states = [wpool.tile([128, 6, 1024], SDT, tag=f"st{b}",
                                 name=f"st{b}") for b in range(BL)]
            xpmax = wpool.tile([128, 34, 34], F32, tag="xpmax")
            xpsum = wpool.tile([128, 34, 34], F32, tag="xpsum")
            rmpad = wpool.tile([128, 34, 32], F32, tag="rmpad")
            rspad = wpool.tile([128, 34, 32], F32, tag="rspad")
            ptmp = [wpool.tile([128, 32, 32], F32, tag=f"ptmp{i}",
                               name=f"ptmp{i}") for i in range(2)]
            nc.gpsimd.memset(xpmax[:], -1e30)
            nc.gpsimd.memset(xpsum[:], 0.0)
            nc.gpsimd.memset(rmpad[:], -1e30)
            nc.gpsimd.memset(rspad[:], 0.0)

            # z-buffers: one shared [128,40,40] geometry, interior [4,36);
            # margins zeroed once and never written again (relu overwrites
            # the full interior on every use). Manual rotation for pipelining.
            zbufs = [wpool.tile([128, 40, 40], F32R, tag=f"z{i}",
                                name=f"z{i}") for i in range(4)]
            for z in zbufs:
                nc.gpsimd.memset(z[:].bitcast(F32), 0.0)
            zctr = [0]

            def flat(ap3):
                return ap3.rearrange("p a b -> p (a b)")

            def relu_into_z(src_ap, scale):
                z = zbufs[zctr[0] % len(zbufs)]
                zctr[0] += 1
                nc.scalar.activation(z[:, 4:36, 4:36], src_ap, ACTF.Relu,
                                     scale=scale)
                return z

            def mm_chunks(psum3, lhsT, rhs3, flags):
                s0, e0, s1, e1 = flags
                nc.tensor.matmul(psum3[:, 0:16, :], lhsT, rhs3[:, 0:16, :],
                                 start=s0, stop=e0)
                nc.tensor.matmul(psum3[:, 16:32, :], lhsT, rhs3[:, 16:32, :],
                                 start=s1, stop=e1)

            def dw_chain(z, dslot, tap0, k, pad, stride):
                dwacc = dpool.tile([128, 32, 32], F32R, tag="dwacc")
                first = True
                for ky in range(k):
                    for kx in range(k):
                        t = tap0 + ky * k + kx
                        y0 = 4 - pad + stride * ky
                        x0 = 4 - pad + stride * kx
                        view = z[:, y0:y0 + 32, x0:x0 + 32]
                        sc = dwt[:, dslot, t:t + 1]
                        if first:
                            nc.vector.tensor_scalar_mul(dwacc[:], view, sc)
                            first = False
                        else:
                            nc.vector.scalar_tensor_tensor(
                                dwacc[:], view, sc, dwacc[:],
                                op0=ALU.mult, op1=ALU.add)
                return dwacc

            def fused_stage(stage_i, z, pad, stride, psum3, gfirst, glast):
                for (a, e) in ((0, 13), (13, 25)):
                    fwt = fwpool.tile([128, 13, 128], F32R, tag="fw")
                    nc.sync.dma_start(fwt[:, 0:e - a, :], fw_d[:, stage_i, a:e, :])
                    for t in range(a, e):
                        ky, kx = divmod(t, 5)
                        y0 = 4 - pad + stride * ky
                        x0 = 4 - pad + stride * kx
                        st = gfirst and t == 0
                        sp = glast and t == 24
                        for h2 in range(2):
                            nc.tensor.matmul(
                                psum3[:, 16 * h2:16 * h2 + 16, :],
                                fwt[:, t - a, :],
                                z[:, y0 + 16 * h2:y0 + 16 * h2 + 16,
                                  x0:x0 + 32],
                                start=st, stop=sp)

            def stream_pw(pw_i):
                t = pwpool.tile([128, 1, 128], F32R, tag="pwb")
                nc.sync.dma_start(t[:], pw_d[:, pw_i:pw_i + 1, :])
                return t[:, 0, :]

            def conv_slot(o, x_ap, cs, ctr, stp, gfirst, glast):
                """Emit conv slot type o reading gathered input x_ap."""
                k, pad, stride = CONV_GEO[o]
                a_ap = alf[:, cs:cs + 1]
                if o in PE_FUSED:
                    z1 = relu_into_z(x_ap, a_ap)
                    if o in TWO_STAGE:
                        scr = scrpool.tile([128, 32, 32], F32, tag="scr")
                        fused_stage(ctr["stage"], z1, pad, stride, scr,
                                    True, True)
                        ctr["stage"] += 1
                        z2 = relu_into_z(scr[:], 1.0)
                        fused_stage(ctr["stage"], z2, pad, stride, stp,
                                    gfirst, glast)
                        ctr["stage"] += 1
                    else:
                        fused_stage(ctr["stage"], z1, pad, stride, stp,
                                    gfirst, glast)
                        ctr["stage"] += 1
                else:
                    z1 = relu_into_z(x_ap, a_ap)
                    dwacc = dw_chain(z1, ctr["dve"], 0, k, pad, stride)
                    if o in TWO_STAGE:
                        scr = scrpool.tile([128, 32, 32], F32, tag="scr")
                        mm_chunks(scr, stream_pw(ctr["pw"]), dwacc,
                                  (True, True, True, True))
                        ctr["pw"] += 1
                        z2 = relu_into_z(scr[:], 1.0)
                        dwacc2 = dw_chain(z2, ctr["dve"], 9, k, pad, stride)
                        mm_chunks(stp, stream_pw(ctr["pw"]), dwacc2,
                                  (gfirst, glast, gfirst, glast))
                        ctr["pw"] += 1
                    else:
                        mm_chunks(stp, stream_pw(ctr["pw"]), dwacc,
                                  (gfirst, glast, gfirst, glast))
                        ctr["pw"] += 1
                    ctr["dve"] += 1

            def cheap_slot(o, x_ap, cc, stp):
                sc = alfc[:, cc:cc + 1]
                if o == O_SKIP:
                    nc.vector.scalar_tensor_tensor(
                        stp[:], x_ap, sc, stp[:], op0=ALU.mult, op1=ALU.add)
                elif o == O_MAX:
                    nc.scalar.copy(xpmax[:, 1:33, 1:33], x_ap)
                    t = ptmp[0]
                    nc.vector.tensor_max(t[:], xpmax[:, 1:33, 0:32],
                                         xpmax[:, 1:33, 1:33])
                    nc.vector.tensor_max(rmpad[:, 1:33, :], t[:],
                                         xpmax[:, 1:33, 2:34])
                    nc.vector.tensor_max(t[:], rmpad[:, 0:32, :],
                                         rmpad[:, 1:33, :])
                    nc.vector.tensor_max(t[:], t[:], rmpad[:, 2:34, :])
                    nc.vector.scalar_tensor_tensor(
                        stp[:], t[:], sc, stp[:], op0=ALU.mult, op1=ALU.add)
                else:  # O_AVG
                    nc.scalar.copy(xpsum[:, 1:33, 1:33], x_ap)
                    t = ptmp[1]
                    nc.gpsimd.tensor_add(t[:], xpsum[:, 1:33, 0:32],
                                         xpsum[:, 1:33, 1:33])
                    nc.gpsimd.tensor_add(rspad[:, 1:33, :], t[:],
                                         xpsum[:, 1:33, 2:34])
                    nc.gpsimd.tensor_add(t[:], rspad[:, 0:32, :],
                                         rspad[:, 1:33, :])
                    nc.gpsimd.tensor_add(t[:], t[:], rspad[:, 2:34, :])
                    nc.gpsimd.tensor_mul(t[:], t[:], rmap[:])
                    nc.vector.scalar_tensor_tensor(
                        stp[:], t[:], sc, stp[:], op0=ALU.mult, op1=ALU.add)

            # ---- preprocess all samples ----
            for bs in range(BL):
                for inp, xd in ((0, x0_d), (1, x1_d)):
                    scr = scrpool.tile([128, 32, 32], F32, tag="scr")
                    for kc in range(4):
                        xb = xpool.tile([128, HW], F32, tag="xb")
                        nc.sync.dma_start(xb[:], xd[bs, kc])
                        xr = xpool.tile([128, HW], F32R, tag="xr")
                        nc.scalar.activation(xr[:], xb[:], ACTF.Relu)
                        for h in range(2):
                            nc.tensor.matmul(
                                scr[:, 16 * h:16 * (h + 1), :],
                                prew[:, inp, kc, :],
                                xr[:, 512 * h:512 * (h + 1)].rearrange(
                                    "p (a c) -> p a c", a=16),
                                start=(kc == 0), stop=(kc == 3))
                    nc.scalar.copy(states[bs][:, inp].rearrange(
                        "p (h w) -> p h w", h=32), scr[:])

            # ---- steps / slots ----
            ctr = dict(stage=0, pw=0, dve=0)
            n_conv_c = 0
            n_cheap_c = 0
            n_wave_c = 0
            for s in range(STEPS):
                for bs in range(BL):
                    convs, cheaps = sched[(bs, s)]
                    n_slots = len(convs) + len(cheaps)
                    n_waves = (n_slots + 1) // 2
                    # gather inputs: waves of 2 slots, d=128 chunks (8/state).
                    # Emitted lazily just before the first consumer so pool-
                    # buffer reuse WAR deps see all prior readers.
                    stgs = {}

                    def slot_x(i):  # [128, 32, 32] view of slot i's input
                        wv = i // 2
                        if wv not in stgs:
                            stg = spool.tile([128, 16, 128], SDT, tag="stg")
                            nc.gpsimd.ap_gather(
                                flat(stg[:]), flat(states[bs][:]),
                                idx[:, n_wave_c + wv:n_wave_c + wv + 1],
                                channels=128, num_elems=48, d=128,
                                num_idxs=16)
                            stgs[wv] = stg
                        stg = stgs[wv]
                        half = stg[:, 8 * (i % 2):8 * (i % 2) + 8, :]
                        return flat(half).rearrange("p (h w) -> p h w", h=32)

                    stp = pspool.tile([128, 32, 32], F32, tag="stp")
                    n_mm = len(convs)
                    for i, o in enumerate(convs):
                        conv_slot(o, slot_x(i), n_conv_c + i, ctr, stp,
                                  i == 0, i == n_mm - 1)
                    n_conv_c += len(convs)
                    if n_mm == 0:
                        nc.vector.memset(stp[:], 0.0)
                    for i, o in enumerate(cheaps):
                        cheap_slot(o, slot_x(len(convs) + i),
                                   n_cheap_c + i, stp)
                    n_cheap_c += len(cheaps)
                    n_wave_c += n_waves
                    # evacuate: bf16 state + f32 output
                    nc.scalar.copy(states[bs][:, 2 + s].rearrange(
                        "p (h w) -> p h w", h=32), stp[:])
                    ost = opool.tile([128, 32, 32], F32, tag="ost")
                    nc.scalar.copy(ost[:], stp[:])
                    nc.sync.dma_start(out_d[bs, s], flat(ost[:]))

    nc.compile()
    return nc


# ---------------------------------------------------------------- host data

def host_prepare(inputs):
    s0, s1 = np.asarray(inputs["s0"]), np.asarray(inputs["s1"])
    gates = np.asarray(inputs["gates"])
    top = int(inputs["top"])
    plan = build_plan(gates, top)
    p, assign, sched, n = plan["p"], plan["assign"], plan["sched"], plan["n"]

    prew = np.empty((128, 2, 4, 128), np.float32)
    for inp, wname in ((0, "pre0_w"), (1, "pre1_w")):
        wmat = np.asarray(inputs[wname]) * BN_SCALE
        for kc in range(4):
            prew[:, inp, kc, :] = wmat[:, 128 * kc:128 * (kc + 1)].T

    # fused fw per (m, stage) for PE branches: fw[ci, t, co] = pw[co,ci]*dw[ci,t]
    def fuse(pw_key, dw_key, m):
        pwm = np.asarray(inputs[pw_key])[m].astype(np.float32) * BN_SCALE
        dwm = np.asarray(inputs[dw_key])[m].astype(np.float32).reshape(C, 25)
        return pwm.T[:, None, :] * dwm[:, :, None]  # [ci, 25, co]

    dw_np = {k: np.asarray(inputs[k]).astype(np.float32)
             for k in ("sep3_dw1", "sep3_dw2", "dil3_dw")}
    pw_np = {k: np.asarray(inputs[k]).astype(np.float32)
             for k in ("sep3_pw1", "sep3_pw2", "dil3_pw")}

    cnt = np.zeros((32, 32), np.float32)
    for dy in (-1, 0, 1):
        for dx in (-1, 0, 1):
            cnt[max(0, dy):32 - max(0, -dy),
                max(0, dx):32 - max(0, -dx)] += 1
    rmap = np.broadcast_to((BN_SCALE / cnt).astype(np.float32),
                           (128, 32, 32)).copy()

    act = plan["act"]
    in_maps = []
    for core in range(N_CORES):
        samples = assign[core]
        fw = np.zeros((128, n["stage"], 25, 128), np.float32)
        pw = np.zeros((128, n["pw"], 128), np.float32)
        dwt = np.zeros((128, n["dve"], 18), np.float32)
        alf_t = np.zeros((n["conv"],), np.float32)
        alfc_t = np.zeros((n["cheap"],), np.float32)
        idx_t = np.zeros((128, n["wave"]), np.int16)
        ns = dict(stage=0, pw=0, dve=0, conv=0, cheap=0, wave=0)

        for s in range(STEPS):
            for bs in range(BL):
                smp = samples[bs]
                convs, cheaps = sched[(bs, s)]
                # actual active items of this core at (bs, s), by type
                items = {o: [] for o in CONV_OPS + CHEAP_OPS}
                for j in range(2 + s):
                    m = OFFSETS[s] + j
                    for o in CONV_OPS + CHEAP_OPS:
                        if act[m, smp, o]:
                            items[o].append((m, j))
                used = {o: 0 for o in items}
                slot_js = []
                for o in convs:
                    if used[o] < len(items[o]):
                        m, j = items[o][used[o]]
                        used[o] += 1
                        a = float(p[m, smp, o])
                    else:
                        m, j, a = None, 0, 0.0
                    slot_js.append(j)
                    ci = ns["conv"]
                    alf_t[ci] = a
                    if o == O_SEP5:
                        if m is not None:
                            fw[:, ns["stage"]] = fuse("sep5_pw1", "sep5_dw1", m)
                            fw[:, ns["stage"] + 1] = fuse("sep5_pw2", "sep5_dw2", m)
                        ns["stage"] += 2
                    elif o == O_DIL5:
                        if m is not None:
                            fw[:, ns["stage"]] = fuse("dil5_pw", "dil5_dw", m)
                        ns["stage"] += 1
                    elif o == O_SEP3:
                        if m is not None:
                            dwt[:, ns["dve"], 0:9] = dw_np["sep3_dw1"][m].reshape(C, 9)
                            dwt[:, ns["dve"], 9:18] = dw_np["sep3_dw2"][m].reshape(C, 9)
                            pw[:, ns["pw"]] = pw_np["sep3_pw1"][m].T * BN_SCALE
                            pw[:, ns["pw"] + 1] = pw_np["sep3_pw2"][m].T * BN_SCALE
                        ns["dve"] += 1
                        ns["pw"] += 2
                    else:  # O_DIL3
                        if m is not None:
                            dwt[:, ns["dve"], 0:9] = dw_np["dil3_dw"][m].reshape(C, 9)
                            pw[:, ns["pw"]] = pw_np["dil3_pw"][m].T * BN_SCALE
                        ns["dve"] += 1
                        ns["pw"] += 1
                    ns["conv"] += 1
                for o in cheaps:
                    if used[o] < len(items[o]):
                        m, j = items[o][used[o]]
                        used[o] += 1
                        a = float(p[m, smp, o])
                    else:
                        m, j, a = None, 0, 0.0
                    slot_js.append(j)
                    if o == O_MAX:
                        a *= BN_SCALE
                    alfc_t[ns["cheap"]] = a
                    ns["cheap"] += 1
                # waves of 2 slots -> idx columns
                for wv in range((len(slot_js) + 1) // 2):
                    j1 = slot_js[2 * wv]
                    j2 = slot_js[2 * wv + 1] if 2 * wv + 1 < len(slot_js) else 0
                    vals = np.concatenate([8 * j1 + np.arange(8),
                                           8 * j2 + np.arange(8)])
                    idx_t[:, ns["wave"]] = vals[np.arange(128) % 16]
                    ns["wave"] += 1

        in_maps.append({
            "x0": s0[samples].reshape(BL, 4, 128, HW).astype(np.float32),
            "x1": s1[samples].reshape(BL, 4, 128, HW).astype(np.float32),
            "prew": prew, "fw": fw, "pw": pw, "dwt": dwt,
            "alf": np.broadcast_to(alf_t, (128, n["conv"])).copy(),
            "alfc": np.broadcast_to(alfc_t, (128, n["cheap"])).copy(),
            "idx": idx_t, "rmap": rmap,
        })
    return in_maps, plan


_prog_cache = {}


def _get_program(plan):
    key = plan["key"]
    if key not in _prog_cache:
        _prog_cache[key] = build_program(plan)
    return _prog_cache[key]


def prepare_run(inputs):
    in_maps, plan = host_prepare(inputs)
    return in_maps, _get_program(plan)


def kernel(**inputs):
    in_maps, plan = host_prepare(inputs)
    nc = _get_program(plan)
    res = run_bass_kernel_spmd(nc, in_maps, core_ids=list(range(N_CORES)))
    out = np.empty((B, 512, H, W), np.float32)
    for core in range(N_CORES):
        o = res.results[core]["out"]  # [BL, 4, 128, HW]
        for bs in range(BL):
            out[plan["assign"][core][bs]] = (
                o[bs].reshape(512, H, W).astype(np.float32))
    return out
